# revision 9
# baseline (speedup 1.0000x reference)
"""Token-choice MoE (8 experts, top-2, SwiGLU) on 8 Trainium2 NeuronCores.

Strategy: expert-parallel. Host computes the (tiny) router matmul + top-2
selection exactly as the reference does (jax on CPU), gathers each expert's
tokens, and ships core e the tokens routed to expert e plus that expert's
weights (pre-transposed/tiled for the TensorEngine, in fp16). Each core runs
the dense SwiGLU FFN over its token batch (fp16 operands, fp32 PSUM
accumulation, ~1e-3 relative error). Host scales per-token by the top-2
router weights and scatter-adds the per-expert outputs back together.

Capacity: each core's main stage covers MAIN_C tokens (4 passes of 512).
Experts routed more than MAIN_C tokens spill into remainder blocks in the
same program: for each overflowing expert, ALL cores process that expert's
spill tokens (padded to a 128 multiple) with the expert's weights sharded
8-ways along the intermediate dim; the host sums the 8 partial outputs.
Remainder block b's input DMAs are issued at the top of main pass b (a full
pass of prefetch time) and its compute runs right after pass b, so the
TensorEngine never waits on remainder data.

Output stores are issued from the scalar engine's HWDGE ring so a blocked
store never stalls input prefetch on the sync ring. x DMAs are chunked along
the contraction dim so the first matmul can start after ~1MB instead of the
full pass tile.

All heavy FLOPs (~825 GFLOP of matmul) run on the device; host work is the
router (0.03% of FLOPs), gathers and the final combine.
"""

import os
import numpy as np

import concourse.bass as bass
import concourse.tile as tile
from concourse import bacc, mybir
from concourse import bass_utils

# Problem geometry (hardcoded per spec)
HIDDEN = 2048
INTER = 4096
N_EXPERTS = 8
TOPK = 2
KT = HIDDEN // 128     # 16 contraction tiles for the first matmul
MT = 2 * INTER // 128  # 64 row tiles of w1 (32 gate + 32 up)
IT = INTER // 128      # 32 contraction tiles for the second matmul
HT = HIDDEN // 128     # 16 output row tiles

NT = 512               # tokens per main pass (moving-operand width)
MAIN_P = 4             # main passes
MAIN_C = MAIN_P * NT   # main-stage capacity per expert
RIT = IT // 8          # remainder stage: i-tiles per core (inter/8)
RMT = RIT * 2          # remainder w1 row tiles per core (gate+up slices)
HG = HT // 4           # remainder w2 column groups (512 rows each)
XCH = 4                # k-dim chunks per x DMA (earlier first-matmul start)

F16 = np.float16

_cache = {}


def _build(rns):
    """Build + compile the SPMD per-core program.

    rns: tuple of remainder block sizes (tokens, each a multiple of 128).
    """
    f16 = mybir.dt.float16
    f32 = mybir.dt.float32
    R = len(rns)

    nc = bacc.Bacc("TRN2", target_bir_lowering=False, debug=False, num_devices=8)
    xt = nc.dram_tensor("xt", [128, KT, MAIN_C], f16, kind="ExternalInput").ap()
    w1t = nc.dram_tensor("w1t", [MT, 128, KT, 128], f16, kind="ExternalInput").ap()
    w2t = nc.dram_tensor("w2t", [HT, 128, IT, 128], f16, kind="ExternalInput").ap()
    zt = nc.dram_tensor("zt", [HT, 128, MAIN_C], f16, kind="ExternalOutput").ap()
    xr = wr1 = wr2 = zr = None
    if R:
        # Per-block tensors (blocks can have different token counts)
        xr = [nc.dram_tensor(f"xr{b}", [128, KT, rns[b]], f16,
                             kind="ExternalInput").ap() for b in range(R)]
        wr1 = nc.dram_tensor("wr1", [R, RMT, 128, KT, 128], f16,
                             kind="ExternalInput").ap()
        wr2 = nc.dram_tensor("wr2", [R, HG, 128, RIT, 512], f16,
                             kind="ExternalInput").ap()
        zr = [nc.dram_tensor(f"zr{b}", [128, rns[b] // 128, HG, 512], f16,
                             kind="ExternalOutput").ap() for b in range(R)]

    with tile.TileContext(nc) as tc:
        with (
            tc.tile_pool(name="xt", bufs=2) as xt_pool,
            tc.tile_pool(name="w1", bufs=6) as w1_pool,
            tc.tile_pool(name="w2", bufs=3) as w2_pool,
            tc.tile_pool(name="hm", bufs=1) as hm_pool,
            tc.tile_pool(name="sg", bufs=2) as sg_pool,
            tc.tile_pool(name="out", bufs=3) as out_pool,
            tc.tile_pool(name="xr", bufs=2) as xr_pool,
            tc.tile_pool(name="wr1", bufs=8) as wr1_pool,
            tc.tile_pool(name="wr2", bufs=8) as wr2_pool,
            tc.tile_pool(name="zrb", bufs=2) as zrb_pool,
            tc.tile_pool(name="ps1", bufs=3, space="PSUM") as ps1,
            tc.tile_pool(name="ps2", bufs=3, space="PSUM") as ps2,
        ):
            def rem_dma(b):
                """Issue remainder block b's input DMAs (prefetch)."""
                rn = rns[b]
                xr_t = xr_pool.tile([128, KT, rn], f16, tag="xr", name=f"xr_t{b}")
                nc.gpsimd.dma_start(xr_t[:], xr[b])
                wgs = []
                for m in range(RMT):
                    wg = wr1_pool.tile([128, KT, 128], f16, tag="wr1",
                                       name=f"wr_t{b}_{m}")
                    nc.gpsimd.dma_start(wg[:], wr1[b, m])
                    wgs.append(wg)
                w2s = []
                for hg in range(HG):
                    w2_t = wr2_pool.tile([128, RIT, 512], f16, tag="wr2",
                                         name=f"wr2_t{b}_{hg}")
                    nc.gpsimd.dma_start(w2_t[:], wr2[b, hg])
                    w2s.append(w2_t)
                return xr_t, wgs, w2s

            def rem_compute(b, tiles):
                rn = rns[b]
                xr_t, wgs, w2s = tiles
                hmr = hm_pool.tile([128, RIT, rn], f16, tag="hm",
                                   name=f"hmr{b}")
                for m in range(RIT):
                    pg = ps1.tile([128, rn], f32, tag="pg", name=f"rpg{b}_{m}")
                    pu = ps1.tile([128, rn], f32, tag="pu", bufs=2, name=f"rpu{b}_{m}")
                    for k in range(KT):
                        nc.tensor.matmul(pg[:], wgs[m][:, k, :], xr_t[:, k, :],
                                         start=(k == 0), stop=(k == KT - 1))
                    for k in range(KT):
                        nc.tensor.matmul(pu[:], wgs[m + RIT][:, k, :],
                                         xr_t[:, k, :],
                                         start=(k == 0), stop=(k == KT - 1))
                    sg = sg_pool.tile([128, rn], f16, tag="sg", name=f"rsg{b}_{m}")
                    nc.scalar.activation(sg[:], pg[:],
                                         mybir.ActivationFunctionType.Silu)
                    nc.vector.tensor_mul(hmr[:, m, :], pu[:], sg[:])
                ntb = rn // 128
                zrb = zrb_pool.tile([128, ntb, HG, 512], f16, tag="zrb",
                                    name=f"zrb{b}")
                for hg in range(HG):
                    for tb in range(ntb):
                        pz = ps2.tile([128, 512], f32, tag="pz",
                                      name=f"rpz{b}_{hg}_{tb}")
                        for i in range(RIT):
                            nc.tensor.matmul(
                                pz[:], hmr[:, i, tb * 128:(tb + 1) * 128],
                                w2s[hg][:, i, :],
                                start=(i == 0), stop=(i == RIT - 1))
                        nc.scalar.copy(zrb[:, tb, hg, :], pz[:])
                nc.scalar.dma_start(zr[b], zrb[:])

            rem_tiles = {}
            xt_tiles = {}

            def xt_dma(p):
                xt_t = xt_pool.tile([128, KT, NT], f16, tag="xt",
                                    name=f"xt_t{p}")
                kc = KT // XCH
                for c in range(XCH):
                    nc.scalar.dma_start(
                        xt_t[:, c * kc:(c + 1) * kc, :],
                        xt[:, c * kc:(c + 1) * kc, p * NT:(p + 1) * NT])
                xt_tiles[p] = xt_t

            xt_dma(0)
            for p in range(MAIN_P):
                xt_t = xt_tiles.pop(p)
                hm = hm_pool.tile([128, IT, NT], f16, tag="hm", name=f"hm{p}")
                # First matmul + SwiGLU: pair gate row-tile m with up tile m+IT
                for m in range(IT):
                    wg = w1_pool.tile([128, KT, 128], f16, tag="w1",
                                      name=f"wg{p}_{m}")
                    nc.sync.dma_start(wg[:], w1t[m])
                    wu = w1_pool.tile([128, KT, 128], f16, tag="w1",
                                      name=f"wu{p}_{m}")
                    nc.sync.dma_start(wu[:], w1t[m + IT])
                    pg = ps1.tile([128, NT], f32, tag="pg", name=f"pg{p}_{m}")
                    pu = ps1.tile([128, NT], f32, tag="pu", bufs=2, name=f"pu{p}_{m}")
                    for k in range(KT):
                        nc.tensor.matmul(pg[:], wg[:, k, :], xt_t[:, k, :],
                                         start=(k == 0), stop=(k == KT - 1))
                    for k in range(KT):
                        nc.tensor.matmul(pu[:], wu[:, k, :], xt_t[:, k, :],
                                         start=(k == 0), stop=(k == KT - 1))
                    sg = sg_pool.tile([128, NT], f16, tag="sg", name=f"sg{p}_{m}")
                    nc.scalar.activation(sg[:], pg[:],
                                         mybir.ActivationFunctionType.Silu)
                    nc.vector.tensor_mul(hm[:, m, :], pu[:], sg[:])
                    if m == 7 and p < R:
                        rem_tiles[p] = rem_dma(p)
                    if m == 23 and p + 1 < MAIN_P:
                        xt_dma(p + 1)
                # Second matmul: z.T[h] = sum_i w2t[h][:, i, :].T @ hm[:, i, :]
                for h in range(HT):
                    w2_t = w2_pool.tile([128, IT, 128], f16, tag="w2",
                                        name=f"w2_t{p}_{h}")
                    nc.sync.dma_start(w2_t[:], w2t[h])
                    pz = ps2.tile([128, NT], f32, tag="pz", name=f"pz{p}_{h}")
                    for i in range(IT):
                        nc.tensor.matmul(pz[:], w2_t[:, i, :], hm[:, i, :],
                                         start=(i == 0), stop=(i == IT - 1))
                    ot = out_pool.tile([128, NT], f16, tag="ot", name=f"ot{p}_{h}")
                    nc.scalar.copy(ot[:], pz[:])
                    nc.scalar.dma_start(zt[h, :, p * NT:(p + 1) * NT], ot[:])
                if p < R:
                    rem_compute(p, rem_tiles.pop(p))
            # Any blocks beyond MAIN_P (extreme imbalance): run at the end.
            for b in range(MAIN_P, R):
                rem_compute(b, rem_dma(b))
    nc.compile()
    return nc


def _tile_x(xg):
    """[C, H] fp32 -> [128, KT, C] fp16 (partition = k % 128, free = (k//128, t))."""
    C = xg.shape[0]
    return np.ascontiguousarray(
        xg.T.reshape(KT, 128, C).transpose(1, 0, 2), dtype=F16)


def kernel(hidden_states, w1, w2, router_w):
    import jax
    import jax.numpy as jnp

    orig_shape = hidden_states.shape
    x = np.ascontiguousarray(np.asarray(hidden_states).reshape(-1, HIDDEN),
                             dtype=np.float32)
    w1 = np.asarray(w1, dtype=np.float32)
    w2 = np.asarray(w2, dtype=np.float32)
    router_w = np.asarray(router_w, dtype=np.float32)

    # Router on CPU, matching the reference ops exactly (jax CPU backend).
    cpu = jax.devices("cpu")[0]
    with jax.default_device(cpu):
        logits = jnp.asarray(x) @ jnp.asarray(router_w).T
        probs = jax.nn.softmax(logits.astype(jnp.float32), axis=-1)
        topk_w, sel = jax.lax.top_k(probs, TOPK)
    topk_w = np.asarray(topk_w)
    sel = np.asarray(sel)

    # Per-expert token lists and combine weights
    idxs, wts = [], []
    for e in range(N_EXPERTS):
        mask = sel == e
        tok = np.nonzero(mask.any(axis=1))[0]
        we = (topk_w * mask).sum(axis=1)[tok].astype(np.float32)
        idxs.append(tok)
        wts.append(we)

    # Remainder blocks: experts with more than MAIN_C tokens spill into
    # blocks of up to 512 tokens, padded to a 128 multiple.
    blocks = []  # (expert, start_offset_in_spill, ntokens, padded)
    for e in range(N_EXPERTS):
        spill = len(idxs[e]) - MAIN_C
        off = 0
        while spill > 0:
            n = min(spill, 512)
            blocks.append((e, MAIN_C + off, n, ((n + 127) // 128) * 128))
            off += n
            spill -= n
    R = len(blocks)
    rns = tuple(b[3] for b in blocks)

    if rns not in _cache:
        _cache[rns] = _build(rns)
    nc = _cache[rns]

    # Per-core inputs: gathered+transposed tokens, tiled weights (fp16)
    in_maps = []
    for e in range(N_EXPERTS):
        tok = idxs[e][:MAIN_C]
        pad = np.zeros(MAIN_C, dtype=np.int64)
        pad[:len(tok)] = tok
        in_map = {
            "xt": _tile_x(x[pad]),
            "w1t": np.ascontiguousarray(
                w1[e].reshape(MT, 128, KT, 128).transpose(0, 3, 2, 1),
                dtype=F16),
            "w2t": np.ascontiguousarray(
                w2[e].reshape(HT, 128, IT, 128).transpose(0, 3, 2, 1),
                dtype=F16),
        }
        if R:
            c = e  # this core's inter-dim shard index
            wr1_l, wr2_l = [], []
            for b, (eb, boff, bn, bpad) in enumerate(blocks):
                rtok = idxs[eb][boff:boff + bn]
                rpad = np.zeros(bpad, dtype=np.int64)
                rpad[:bn] = rtok
                in_map[f"xr{b}"] = _tile_x(x[rpad])
                w1e = w1[eb].reshape(MT, 128, KT, 128).transpose(0, 3, 2, 1)
                gsel = w1e[c * RIT:(c + 1) * RIT]
                usel = w1e[IT + c * RIT:IT + (c + 1) * RIT]
                wr1_l.append(np.concatenate([gsel, usel], axis=0))
                w2e = w2[eb].reshape(HT, 128, IT, 128).transpose(0, 3, 2, 1)
                w2s_ = w2e[:, :, c * RIT:(c + 1) * RIT, :]
                wr2_l.append(
                    w2s_.reshape(HG, 4, 128, RIT, 128)
                    .transpose(0, 2, 3, 1, 4).reshape(HG, 128, RIT, 512))
            in_map["wr1"] = np.ascontiguousarray(np.stack(wr1_l), dtype=F16)
            in_map["wr2"] = np.ascontiguousarray(np.stack(wr2_l), dtype=F16)
        in_maps.append(in_map)

    trace = bool(int(os.environ.get("MOE_KERNEL_TRACE", "0")))
    res = bass_utils.run_bass_kernel_spmd(
        nc, in_maps, core_ids=list(range(8)), trace=trace)
    kernel.last_exec_time_ns = res.exec_time_ns
    kernel.last_results = res

    out = np.zeros_like(x)
    for e in range(N_EXPERTS):
        tok = idxs[e][:MAIN_C]
        ztile = np.asarray(res.results[e]["zt"], dtype=np.float32)
        y = ztile.transpose(2, 0, 1).reshape(MAIN_C, HIDDEN)[:len(tok)]
        out[tok] += y * wts[e][:len(tok), None]
    for b, (eb, boff, bn, bpad) in enumerate(blocks):
        rtok = idxs[eb][boff:boff + bn]
        zsum = sum(np.asarray(res.results[c][f"zr{b}"], dtype=np.float32)
                   for c in range(N_EXPERTS))
        y = zsum.reshape(128, bpad // 128, HIDDEN).transpose(1, 0, 2)
        y = y.reshape(bpad, HIDDEN)[:bn]
        out[rtok] += y * wts[eb][boff:boff + bn, None]
    return out.reshape(orig_shape)


# revision 10
# speedup vs baseline: 1.0032x; 1.0032x over previous
"""Token-choice MoE (8 experts, top-2, SwiGLU) on 8 Trainium2 NeuronCores.

Strategy: expert-parallel. Host computes the (tiny) router matmul + top-2
selection exactly as the reference does (jax on CPU), gathers each expert's
tokens, and ships core e the tokens routed to expert e plus that expert's
weights (pre-transposed/tiled for the TensorEngine, in fp16). Each core runs
the dense SwiGLU FFN over its token batch (fp16 operands, fp32 PSUM
accumulation, ~1e-3 relative error). Host scales per-token by the top-2
router weights and scatter-adds the per-expert outputs back together.

Capacity: each core's main stage covers MAIN_C tokens (4 passes of 512).
Experts routed more than MAIN_C tokens spill into remainder blocks in the
same program: for each overflowing expert, ALL cores process that expert's
spill tokens (padded to a 128 multiple) with the expert's weights sharded
8-ways along the intermediate dim; the host sums the 8 partial outputs.
Remainder block b's input DMAs are issued at the top of main pass b (a full
pass of prefetch time) and its compute runs right after pass b, so the
TensorEngine never waits on remainder data.

Output stores are issued from the scalar engine's HWDGE ring so a blocked
store never stalls input prefetch on the sync ring. x DMAs are chunked along
the contraction dim so the first matmul can start after ~1MB instead of the
full pass tile.

All heavy FLOPs (~825 GFLOP of matmul) run on the device; host work is the
router (0.03% of FLOPs), gathers and the final combine.
"""

import os
import numpy as np

import concourse.bass as bass
import concourse.tile as tile
from concourse import bacc, mybir
from concourse import bass_utils

# Problem geometry (hardcoded per spec)
HIDDEN = 2048
INTER = 4096
N_EXPERTS = 8
TOPK = 2
KT = HIDDEN // 128     # 16 contraction tiles for the first matmul
MT = 2 * INTER // 128  # 64 row tiles of w1 (32 gate + 32 up)
IT = INTER // 128      # 32 contraction tiles for the second matmul
HT = HIDDEN // 128     # 16 output row tiles

NT = 512               # tokens per main pass (moving-operand width)
MAIN_P = 4             # main passes
MAIN_C = MAIN_P * NT   # main-stage capacity per expert
RIT = IT // 8          # remainder stage: i-tiles per core (inter/8)
RMT = RIT * 2          # remainder w1 row tiles per core (gate+up slices)
HG = HT // 4           # remainder w2 column groups (512 rows each)
XCH = 4                # k-dim chunks per x DMA (earlier first-matmul start)

F16 = np.float16

_cache = {}


def _build(rns):
    """Build + compile the SPMD per-core program.

    rns: tuple of remainder block sizes (tokens, each a multiple of 128).
    """
    f16 = mybir.dt.float16
    f32 = mybir.dt.float32
    R = len(rns)

    nc = bacc.Bacc("TRN2", target_bir_lowering=False, debug=False, num_devices=8)
    xt = nc.dram_tensor("xt", [128, KT, MAIN_C], f16, kind="ExternalInput").ap()
    w1t = nc.dram_tensor("w1t", [MT, 128, KT, 128], f16, kind="ExternalInput").ap()
    w2t = nc.dram_tensor("w2t", [HT, 128, IT, 128], f16, kind="ExternalInput").ap()
    zt = nc.dram_tensor("zt", [HT, 128, MAIN_C], f16, kind="ExternalOutput").ap()
    xr = wr1 = wr2 = zr = None
    if R:
        # Per-block tensors (blocks can have different token counts)
        xr = [nc.dram_tensor(f"xr{b}", [128, KT, rns[b]], f16,
                             kind="ExternalInput").ap() for b in range(R)]
        wr1 = nc.dram_tensor("wr1", [R, RMT, 128, KT, 128], f16,
                             kind="ExternalInput").ap()
        wr2 = nc.dram_tensor("wr2", [R, HG, 128, RIT, 512], f16,
                             kind="ExternalInput").ap()
        zr = [nc.dram_tensor(f"zr{b}", [128, rns[b] // 128, HG, 512], f16,
                             kind="ExternalOutput").ap() for b in range(R)]

    with tile.TileContext(nc) as tc:
        with (
            tc.tile_pool(name="xt", bufs=2) as xt_pool,
            tc.tile_pool(name="w1", bufs=6) as w1_pool,
            tc.tile_pool(name="w2", bufs=3) as w2_pool,
            tc.tile_pool(name="hm", bufs=1) as hm_pool,
            tc.tile_pool(name="sg", bufs=2) as sg_pool,
            tc.tile_pool(name="out", bufs=3) as out_pool,
            tc.tile_pool(name="xr", bufs=2) as xr_pool,
            tc.tile_pool(name="wr1", bufs=8) as wr1_pool,
            tc.tile_pool(name="wr2", bufs=8) as wr2_pool,
            tc.tile_pool(name="zrb", bufs=2) as zrb_pool,
            tc.tile_pool(name="ps1", bufs=3, space="PSUM") as ps1,
            tc.tile_pool(name="ps2", bufs=3, space="PSUM") as ps2,
        ):
            def rem_dma(b):
                """Issue remainder block b's input DMAs (prefetch)."""
                rn = rns[b]
                xr_t = xr_pool.tile([128, KT, rn], f16, tag="xr", name=f"xr_t{b}")
                nc.gpsimd.dma_start(xr_t[:], xr[b])
                wgs = []
                for m in range(RMT):
                    wg = wr1_pool.tile([128, KT, 128], f16, tag="wr1",
                                       name=f"wr_t{b}_{m}")
                    nc.gpsimd.dma_start(wg[:], wr1[b, m])
                    wgs.append(wg)
                w2s = []
                for hg in range(HG):
                    w2_t = wr2_pool.tile([128, RIT, 512], f16, tag="wr2",
                                         name=f"wr2_t{b}_{hg}")
                    nc.gpsimd.dma_start(w2_t[:], wr2[b, hg])
                    w2s.append(w2_t)
                return xr_t, wgs, w2s

            def rem_compute(b, tiles):
                rn = rns[b]
                xr_t, wgs, w2s = tiles
                hmr = hm_pool.tile([128, RIT, rn], f16, tag="hm",
                                   name=f"hmr{b}")
                for m in range(RIT):
                    pg = ps1.tile([128, rn], f32, tag="pg", name=f"rpg{b}_{m}")
                    pu = ps1.tile([128, rn], f32, tag="pu", bufs=2, name=f"rpu{b}_{m}")
                    for k in range(KT):
                        nc.tensor.matmul(pg[:], wgs[m][:, k, :], xr_t[:, k, :],
                                         start=(k == 0), stop=(k == KT - 1))
                    for k in range(KT):
                        nc.tensor.matmul(pu[:], wgs[m + RIT][:, k, :],
                                         xr_t[:, k, :],
                                         start=(k == 0), stop=(k == KT - 1))
                    sg = sg_pool.tile([128, rn], f16, tag="sg", name=f"rsg{b}_{m}")
                    nc.scalar.activation(sg[:], pg[:],
                                         mybir.ActivationFunctionType.Silu)
                    nc.vector.tensor_mul(hmr[:, m, :], pu[:], sg[:])
                ntb = rn // 128
                zrb = zrb_pool.tile([128, ntb, HG, 512], f16, tag="zrb",
                                    name=f"zrb{b}")
                for hg in range(HG):
                    for tb in range(ntb):
                        pz = ps2.tile([128, 512], f32, tag="pz",
                                      name=f"rpz{b}_{hg}_{tb}")
                        for i in range(RIT):
                            nc.tensor.matmul(
                                pz[:], hmr[:, i, tb * 128:(tb + 1) * 128],
                                w2s[hg][:, i, :],
                                start=(i == 0), stop=(i == RIT - 1))
                        nc.scalar.copy(zrb[:, tb, hg, :], pz[:])
                nc.scalar.dma_start(zr[b], zrb[:])

            rem_tiles = {}
            for p in range(MAIN_P):
                xt_t = xt_pool.tile([128, KT, NT], f16, tag="xt", name=f"xt_t{p}")
                kc = KT // XCH
                for c in range(XCH):
                    nc.sync.dma_start(
                        xt_t[:, c * kc:(c + 1) * kc, :],
                        xt[:, c * kc:(c + 1) * kc, p * NT:(p + 1) * NT])
                hm = hm_pool.tile([128, IT, NT], f16, tag="hm", name=f"hm{p}")
                # First matmul + SwiGLU: pair gate row-tile m with up tile m+IT
                for m in range(IT):
                    wg = w1_pool.tile([128, KT, 128], f16, tag="w1",
                                      name=f"wg{p}_{m}")
                    nc.sync.dma_start(wg[:], w1t[m])
                    wu = w1_pool.tile([128, KT, 128], f16, tag="w1",
                                      name=f"wu{p}_{m}")
                    nc.sync.dma_start(wu[:], w1t[m + IT])
                    pg = ps1.tile([128, NT], f32, tag="pg", name=f"pg{p}_{m}")
                    pu = ps1.tile([128, NT], f32, tag="pu", bufs=2, name=f"pu{p}_{m}")
                    for k in range(KT):
                        nc.tensor.matmul(pg[:], wg[:, k, :], xt_t[:, k, :],
                                         start=(k == 0), stop=(k == KT - 1))
                    for k in range(KT):
                        nc.tensor.matmul(pu[:], wu[:, k, :], xt_t[:, k, :],
                                         start=(k == 0), stop=(k == KT - 1))
                    sg = sg_pool.tile([128, NT], f16, tag="sg", name=f"sg{p}_{m}")
                    nc.scalar.activation(sg[:], pg[:],
                                         mybir.ActivationFunctionType.Silu)
                    nc.vector.tensor_mul(hm[:, m, :], pu[:], sg[:])
                    if m == 7 and p < R:
                        rem_tiles[p] = rem_dma(p)
                # Second matmul: z.T[h] = sum_i w2t[h][:, i, :].T @ hm[:, i, :]
                for h in range(HT):
                    w2_t = w2_pool.tile([128, IT, 128], f16, tag="w2",
                                        name=f"w2_t{p}_{h}")
                    nc.sync.dma_start(w2_t[:], w2t[h])
                    pz = ps2.tile([128, NT], f32, tag="pz", name=f"pz{p}_{h}")
                    for i in range(IT):
                        nc.tensor.matmul(pz[:], w2_t[:, i, :], hm[:, i, :],
                                         start=(i == 0), stop=(i == IT - 1))
                    ot = out_pool.tile([128, NT], f16, tag="ot", name=f"ot{p}_{h}")
                    nc.scalar.copy(ot[:], pz[:])
                    nc.scalar.dma_start(zt[h, :, p * NT:(p + 1) * NT], ot[:])
                if p < R:
                    rem_compute(p, rem_tiles.pop(p))
            # Any blocks beyond MAIN_P (extreme imbalance): run at the end.
            for b in range(MAIN_P, R):
                rem_compute(b, rem_dma(b))
    nc.compile()
    return nc


def _tile_x(xg):
    """[C, H] fp32 -> [128, KT, C] fp16 (partition = k % 128, free = (k//128, t))."""
    C = xg.shape[0]
    return np.ascontiguousarray(
        xg.T.reshape(KT, 128, C).transpose(1, 0, 2), dtype=F16)


def kernel(hidden_states, w1, w2, router_w):
    import jax
    import jax.numpy as jnp

    orig_shape = hidden_states.shape
    x = np.ascontiguousarray(np.asarray(hidden_states).reshape(-1, HIDDEN),
                             dtype=np.float32)
    w1 = np.asarray(w1, dtype=np.float32)
    w2 = np.asarray(w2, dtype=np.float32)
    router_w = np.asarray(router_w, dtype=np.float32)

    # Router on CPU, matching the reference ops exactly (jax CPU backend).
    cpu = jax.devices("cpu")[0]
    with jax.default_device(cpu):
        logits = jnp.asarray(x) @ jnp.asarray(router_w).T
        probs = jax.nn.softmax(logits.astype(jnp.float32), axis=-1)
        topk_w, sel = jax.lax.top_k(probs, TOPK)
    topk_w = np.asarray(topk_w)
    sel = np.asarray(sel)

    # Per-expert token lists and combine weights
    idxs, wts = [], []
    for e in range(N_EXPERTS):
        mask = sel == e
        tok = np.nonzero(mask.any(axis=1))[0]
        we = (topk_w * mask).sum(axis=1)[tok].astype(np.float32)
        idxs.append(tok)
        wts.append(we)

    # Remainder blocks: experts with more than MAIN_C tokens spill into
    # blocks of up to 512 tokens, padded to a 128 multiple.
    blocks = []  # (expert, start_offset_in_spill, ntokens, padded)
    for e in range(N_EXPERTS):
        spill = len(idxs[e]) - MAIN_C
        off = 0
        while spill > 0:
            n = min(spill, 512)
            blocks.append((e, MAIN_C + off, n, ((n + 127) // 128) * 128))
            off += n
            spill -= n
    R = len(blocks)
    rns = tuple(b[3] for b in blocks)

    if rns not in _cache:
        _cache[rns] = _build(rns)
    nc = _cache[rns]

    # Per-core inputs: gathered+transposed tokens, tiled weights (fp16)
    in_maps = []
    for e in range(N_EXPERTS):
        tok = idxs[e][:MAIN_C]
        pad = np.zeros(MAIN_C, dtype=np.int64)
        pad[:len(tok)] = tok
        in_map = {
            "xt": _tile_x(x[pad]),
            "w1t": np.ascontiguousarray(
                w1[e].reshape(MT, 128, KT, 128).transpose(0, 3, 2, 1),
                dtype=F16),
            "w2t": np.ascontiguousarray(
                w2[e].reshape(HT, 128, IT, 128).transpose(0, 3, 2, 1),
                dtype=F16),
        }
        if R:
            c = e  # this core's inter-dim shard index
            wr1_l, wr2_l = [], []
            for b, (eb, boff, bn, bpad) in enumerate(blocks):
                rtok = idxs[eb][boff:boff + bn]
                rpad = np.zeros(bpad, dtype=np.int64)
                rpad[:bn] = rtok
                in_map[f"xr{b}"] = _tile_x(x[rpad])
                w1e = w1[eb].reshape(MT, 128, KT, 128).transpose(0, 3, 2, 1)
                gsel = w1e[c * RIT:(c + 1) * RIT]
                usel = w1e[IT + c * RIT:IT + (c + 1) * RIT]
                wr1_l.append(np.concatenate([gsel, usel], axis=0))
                w2e = w2[eb].reshape(HT, 128, IT, 128).transpose(0, 3, 2, 1)
                w2s_ = w2e[:, :, c * RIT:(c + 1) * RIT, :]
                wr2_l.append(
                    w2s_.reshape(HG, 4, 128, RIT, 128)
                    .transpose(0, 2, 3, 1, 4).reshape(HG, 128, RIT, 512))
            in_map["wr1"] = np.ascontiguousarray(np.stack(wr1_l), dtype=F16)
            in_map["wr2"] = np.ascontiguousarray(np.stack(wr2_l), dtype=F16)
        in_maps.append(in_map)

    trace = bool(int(os.environ.get("MOE_KERNEL_TRACE", "0")))
    res = bass_utils.run_bass_kernel_spmd(
        nc, in_maps, core_ids=list(range(8)), trace=trace)
    kernel.last_exec_time_ns = res.exec_time_ns
    kernel.last_results = res

    out = np.zeros_like(x)
    for e in range(N_EXPERTS):
        tok = idxs[e][:MAIN_C]
        ztile = np.asarray(res.results[e]["zt"], dtype=np.float32)
        y = ztile.transpose(2, 0, 1).reshape(MAIN_C, HIDDEN)[:len(tok)]
        out[tok] += y * wts[e][:len(tok), None]
    for b, (eb, boff, bn, bpad) in enumerate(blocks):
        rtok = idxs[eb][boff:boff + bn]
        zsum = sum(np.asarray(res.results[c][f"zr{b}"], dtype=np.float32)
                   for c in range(N_EXPERTS))
        y = zsum.reshape(128, bpad // 128, HIDDEN).transpose(1, 0, 2)
        y = y.reshape(bpad, HIDDEN)[:bn]
        out[rtok] += y * wts[eb][boff:boff + bn, None]
    return out.reshape(orig_shape)
